# revision 1
# baseline (speedup 1.0000x reference)
"""Grouped-query attention (2 query heads, 1 pooled KV head) with RoPE,
causal softmax — Trainium2 Bass/Tile kernel, 8 NeuronCores.

Sharding: one core per (batch, head) pair (4 x 2 = 8 cores). The pooled KV
head is head-averaged on the host (mean over heads commutes with the linear
projection), so each core does: q/k/v projections, RoPE on q/k, causal
attention, all in fp32.

Layout strategy (all chosen so every DMA is a natural contiguous slice):
  - host passes x[b] transposed (xT [C, T]) so the contraction dim c lands on
    SBUF partitions for the projection matmuls (x chunks are the stationary
    operand, shared by q/k/v).
  - projections produce q/k/v in natural [t, d] layout; RoPE is applied there.
  - the head dim of q/k (and their weights + rope tables) is permuted to
    "evens-then-odds" (pi) so the RoPE pair swap becomes a contiguous
    half-block swap along the free dim (full-rate DVE, no gathers). The
    permutation cancels in the q.k^T contraction.
  - rotated q/k are PE-transposed to qT/kT [d, t]; scores are computed
    transposed (scoresT [s, t]) so softmax normalization can be folded in
    after the AV matmul, and wei never needs a transpose.
  - softmax skips the max-subtraction (scores are O(1) by construction:
    weights are scale-0.02 gaussians), exp + scale fused into one ScalarE op;
    denominators via ones-vector matmul; 1/sum applied to the AV output.
"""

import sys

sys.path.insert(0, "/opt/trn_rl_repo")

import numpy as np

B, T, C = 4, 2048, 2048
H, D = 2, 512
NCORES = 8
ROPE_THETA = 10000.0
P = 128
NT = T // P  # 16 t/s tiles of 128
NCC = C // P  # 16 contraction chunks of 128
NDT = D // P  # 4 head-dim tiles of 128
TS = 512  # t-slab width for attention phase
NSLAB = T // TS  # 4

_CACHE = {}


def _build():
    import concourse.bass as bass
    import concourse.mybir as mybir
    from concourse import bacc
    from concourse.tile import TileContext

    fp32 = mybir.dt.float32
    EXP = mybir.ActivationFunctionType.Exp

    nc = bacc.Bacc()
    xT = nc.dram_tensor("xT", [C, T], fp32, kind="ExternalInput")
    wq = nc.dram_tensor("wq", [C, D], fp32, kind="ExternalInput")  # pi-permuted
    wk = nc.dram_tensor("wk", [C, D], fp32, kind="ExternalInput")  # pi-permuted, head-avg
    wv = nc.dram_tensor("wv", [C, D], fp32, kind="ExternalInput")  # head-avg
    cosb = nc.dram_tensor("cosb", [T, D], fp32, kind="ExternalInput")
    sinb = nc.dram_tensor("sinb", [T, D], fp32, kind="ExternalInput")
    masks = nc.dram_tensor("masks", [P, 4 * TS], fp32, kind="ExternalInput")
    ident = nc.dram_tensor("ident", [P, P], fp32, kind="ExternalInput")
    ones = nc.dram_tensor("ones", [P, 1], fp32, kind="ExternalInput")
    oT = nc.dram_tensor("oT", [D, T], fp32, kind="ExternalOutput")

    scale = float(D) ** -0.5

    with TileContext(nc) as tc:
        with tc.tile_pool(name="persist", bufs=1) as pp:
            qT_sb = pp.tile([P, NDT, T], fp32)
            kT_sb = pp.tile([P, NDT, T], fp32)
            v_sb = pp.tile([P, NT, D], fp32)

            # ---------------- phase 1: projections + rope + transposes ----
            with (
                tc.tile_pool(name="wpool", bufs=1) as wp,
                tc.tile_pool(name="stream", bufs=2) as sp,
                tc.tile_pool(name="rope", bufs=2) as rp,
                tc.tile_pool(name="ps_proj", bufs=2, space="PSUM") as ps1,
                tc.tile_pool(name="ps_tp", bufs=2, space="PSUM") as pst,
            ):
                ident_sb = wp.tile([P, P], fp32)
                nc.sync.dma_start(out=ident_sb, in_=ident[:, :])
                wq_sb = wp.tile([P, NCC, D], fp32)
                wk_sb = wp.tile([P, NCC, D], fp32)
                wv_sb = wp.tile([P, NCC, D], fp32)
                nc.sync.dma_start(out=wq_sb, in_=wq.rearrange("(cc p) d -> p cc d", p=P))
                nc.sync.dma_start(out=wk_sb, in_=wk.rearrange("(cc p) d -> p cc d", p=P))
                nc.sync.dma_start(out=wv_sb, in_=wv.rearrange("(cc p) d -> p cc d", p=P))
                for tp in range(T // 256):  # 256-wide x pieces, 2 t-tiles each
                    t0 = tp * 256
                    cos_t = [None, None]
                    sin_t = [None, None]
                    for i in range(2):
                        cos_t[i] = sp.tile([P, D], fp32, tag="cos", name="cos_t")
                        sin_t[i] = sp.tile([P, D], fp32, tag="sin", name="sin_t")
                        r0 = t0 + i * P
                        nc.sync.dma_start(out=cos_t[i], in_=cosb[r0 : r0 + P, :])
                        nc.sync.dma_start(out=sin_t[i], in_=sinb[r0 : r0 + P, :])
                    qp = [ps1.tile([P, D], fp32, tag="q", name="qp") for _ in range(2)]
                    kp = [ps1.tile([P, D], fp32, tag="k", name="kp") for _ in range(2)]
                    vp = [ps1.tile([P, D], fp32, tag="v", name="vp") for _ in range(2)]
                    for c in range(NCC):
                        xc = sp.tile([P, 256], fp32, tag="x", bufs=2)
                        nc.sync.dma_start(
                            out=xc, in_=xT[c * P : (c + 1) * P, t0 : t0 + 256]
                        )
                        st, sn = (c == 0), (c == NCC - 1)
                        for i in range(2):
                            xsl = xc[:, i * P : (i + 1) * P]
                            nc.tensor.matmul(
                                qp[i], xsl, wq_sb[:, c, :], start=st, stop=sn
                            )
                            nc.tensor.matmul(
                                kp[i], xsl, wk_sb[:, c, :], start=st, stop=sn
                            )
                            nc.tensor.matmul(
                                vp[i], xsl, wv_sb[:, c, :], start=st, stop=sn
                            )
                    hd = D // 2
                    for i in range(2):
                        tt = tp * 2 + i
                        # v: straight copy to resident natural layout
                        nc.any.tensor_copy(v_sb[:, tt, :], vp[i])
                        for src, dst in ((qp[i], qT_sb), (kp[i], kT_sb)):
                            tmp = rp.tile([P, D], fp32, tag="rtmp", bufs=1)
                            nc.vector.tensor_mul(
                                tmp[:, 0:hd], src[:, hd:D], sin_t[i][:, 0:hd]
                            )
                            nc.vector.tensor_mul(
                                tmp[:, hd:D], src[:, 0:hd], sin_t[i][:, hd:D]
                            )
                            qc = rp.tile([P, D], fp32, tag="rcos", bufs=1)
                            nc.vector.tensor_mul(qc, src, cos_t[i])
                            nc.vector.tensor_add(qc, qc, tmp)
                            tps = pst.tile([P, NDT, P], fp32, tag="tp")
                            for dt in range(NDT):
                                nc.tensor.transpose(
                                    tps[:, dt, :],
                                    qc[:, dt * P : (dt + 1) * P],
                                    ident_sb,
                                )
                            nc.any.tensor_copy(
                                dst[:, :, tt * P : (tt + 1) * P], tps
                            )

            # ---------------- phase 2: causal attention -------------------
            with (
                tc.tile_pool(name="ps_sc", bufs=2, space="PSUM") as pssc,
                tc.tile_pool(name="ps_sum", bufs=2, space="PSUM") as pssum,
                tc.tile_pool(name="ps_av", bufs=4, space="PSUM") as psav,
                tc.tile_pool(name="expp", bufs=3) as ep,
                tc.tile_pool(name="outp", bufs=3) as op_,
            ):
                ones_sb = op_.tile([P, 1], fp32, tag="ones", bufs=1)
                nc.sync.dma_start(out=ones_sb, in_=ones[:, :])
                mask_sb = op_.tile([P, 4 * TS], fp32, tag="masks", bufs=1)
                nc.sync.dma_start(out=mask_sb, in_=masks[:, :])
                for j in range(NSLAB):
                    tsl = slice(j * TS, (j + 1) * TS)
                    nst = (TS // P) * (j + 1)  # s-tiles needed (causal)
                    sums = pssum.tile([1, TS], fp32, tag="sum")
                    av = [psav.tile([P, TS], fp32, tag="av", name="av") for _ in range(NDT)]
                    for st in range(nst):
                        sc = pssc.tile([P, TS], fp32, tag="sc")
                        for dt in range(NDT):
                            nc.tensor.matmul(
                                sc,
                                kT_sb[:, dt, st * P : (st + 1) * P],
                                qT_sb[:, dt, tsl],
                                start=(dt == 0),
                                stop=(dt == NDT - 1),
                            )
                        expt = ep.tile([P, TS], fp32, tag="exp")
                        nc.scalar.activation(
                            out=expt, in_=sc, func=EXP, scale=scale
                        )
                        m = st - (TS // P) * j
                        if m >= 0:  # diagonal tile: zero the s > t half
                            nc.vector.tensor_mul(
                                expt,
                                expt,
                                mask_sb[:, m * TS : (m + 1) * TS],
                            )
                        nc.tensor.matmul(
                            sums,
                            ones_sb,
                            expt,
                            start=(st == 0),
                            stop=(st == nst - 1),
                        )
                        for dt in range(NDT):
                            nc.tensor.matmul(
                                av[dt],
                                v_sb[:, st, dt * P : (dt + 1) * P],
                                expt,
                                start=(st == 0),
                                stop=(st == nst - 1),
                            )
                    rec = op_.tile([1, TS], fp32, tag="rec")
                    nc.vector.reciprocal(rec, sums)
                    recb = op_.tile([P, TS], fp32, tag="recb", bufs=2)
                    nc.gpsimd.partition_broadcast(recb, rec)
                    for dt in range(NDT):
                        ob = op_.tile([P, TS], fp32, tag="ob")
                        nc.vector.tensor_mul(ob, av[dt], recb)
                        nc.sync.dma_start(out=oT[dt * P : (dt + 1) * P, tsl], in_=ob)

    nc.finalize()
    return nc


def _host_inputs(x, Wq, Wk, Wv):
    pi = np.concatenate([np.arange(0, D, 2), np.arange(1, D, 2)])
    f32 = np.float32

    wk_avg = Wk.mean(axis=0)  # [D, C]
    wv_avg = Wv.mean(axis=0)
    wk_p = np.ascontiguousarray(wk_avg.T[:, pi], dtype=f32)
    wv_t = np.ascontiguousarray(wv_avg.T, dtype=f32)

    freqs = 1.0 / (ROPE_THETA ** (np.arange(0, D, 2, dtype=np.float64) / D))
    ang = np.arange(T, dtype=np.float64)[:, None] * freqs[None, :]  # [T, D/2]
    cosb = np.concatenate([np.cos(ang), np.cos(ang)], axis=1).astype(f32)
    sinb = np.concatenate([-np.sin(ang), np.sin(ang)], axis=1).astype(f32)

    m = np.zeros((P, 4 * TS), f32)
    for mi in range(4):
        s = np.arange(P)[:, None] + mi * P
        t = np.arange(TS)[None, :]
        m[:, mi * TS : (mi + 1) * TS] = (s <= t).astype(f32)

    ident = np.eye(P, dtype=f32)
    ones = np.ones((P, 1), f32)

    shared = {
        "wk": wk_p,
        "wv": wv_t,
        "cosb": cosb,
        "sinb": sinb,
        "masks": m,
        "ident": ident,
        "ones": ones,
    }
    in_maps = []
    for i in range(NCORES):
        b, h = i // H, i % H
        in_maps.append(
            {
                "xT": np.ascontiguousarray(x[b].T, dtype=f32),
                "wq": np.ascontiguousarray(Wq[h].T[:, pi], dtype=f32),
                **shared,
            }
        )
    return in_maps


def _run(x, Wq, Wk, Wv, trace=False):
    from concourse.bass_utils import run_bass_kernel_spmd

    if "nc" not in _CACHE:
        _CACHE["nc"] = _build()
    in_maps = _host_inputs(x, Wq, Wk, Wv)
    res = run_bass_kernel_spmd(
        _CACHE["nc"], in_maps, list(range(NCORES)), trace=trace
    )
    out = np.empty((B, H, T, D), np.float32)
    for i in range(NCORES):
        out[i // H, i % H] = res.results[i]["oT"].T
    return out.reshape(B, T, H * D), res


def kernel(**inputs):
    out, _ = _run(inputs["x"], inputs["Wq"], inputs["Wk"], inputs["Wv"])
    return out



# revision 13
# speedup vs baseline: 3.4744x; 3.4744x over previous
"""Grouped-query attention (2 query heads, 1 pooled KV head) with RoPE,
causal softmax — Trainium2 Bass/Tile kernel, 8 NeuronCores.

Sharding: one core per (batch, head) pair (4 x 2 = 8 cores). The pooled KV
head is head-averaged on the host (mean over heads commutes with the linear
projection), so each core does: q/k/v projections, RoPE on q/k, causal
attention.

Precision: all matmuls run in fp32r (tf32) — 1 PE cycle/row instead of
fp32's 4 — with fp32 PSUM accumulation. Inputs are tf32-rounded on the
host; on-device fp32r producers (rope DVE ops, exp activation, PSUM
evacuation copies) round at write time as the BIR verifier requires.

Layout strategy (all chosen so every DMA is a natural contiguous slice):
  - host passes x[b] transposed (xT [C, T]) so the contraction dim c lands on
    SBUF partitions for the projection matmuls (x chunks are the stationary
    operand, shared by q/k/v).
  - projections produce q/k/v in natural [t, d] layout; RoPE is applied there.
  - the head dim of q/k (and their weights) is permuted to "evens-then-odds"
    (pi) so the RoPE pair swap becomes a contiguous half-block swap along the
    free dim. The permutation cancels in the q.k^T contraction. RoPE tables
    are stored halved (cos/sin of [T, D/2]) with the sign handled by
    tensor_sub/tensor_add on the two halves.
  - rotated q/k are PE-transposed (fp32r, exact) to qT/kT [d, t]; scores are
    computed transposed (scoresT [s, t]) so softmax normalization can be
    folded in after the AV matmul, and wei never needs a transpose.
  - softmax skips the max-subtraction (scores are O(1) by construction);
    exp + scale fused into one ScalarE op; denominators via ones-vector
    matmul; 1/sum applied to the AV output.

Engine/queue split (DMAs serialize per issuing engine in the HW queues):
  - SP queue: identity, per-chunk weight stream, phase-2 masks/ones + output.
  - Act queue: x chunk stream + rope tables (+ v and av PSUM evacuations,
    exp activations).
  - DVE: rope math, qT/kT PSUM evacuations, mask muls, 1/sum scaling.
"""

import sys

sys.path.insert(0, "/opt/trn_rl_repo")

import numpy as np

B, T, C = 4, 2048, 2048
H, D = 2, 512
HD = D // 2
NCORES = 8
ROPE_THETA = 10000.0
P = 128
NT = T // P  # 16 t/s tiles of 128
NCC = C // P  # 16 contraction chunks of 128
NDT = D // P  # 4 head-dim tiles of 128
TS = 512  # t-slab width for attention phase
NSLAB = T // TS  # 4

_CACHE = {}


def _build():
    import concourse.bass as bass
    import concourse.mybir as mybir
    from concourse import bacc
    from concourse.tile import TileContext

    fp32 = mybir.dt.float32
    R = mybir.dt.float32r
    EXP = mybir.ActivationFunctionType.Exp

    nc = bacc.Bacc()
    xT = nc.dram_tensor("xT", [C, T], R, kind="ExternalInput")
    wq = nc.dram_tensor("wq", [C, D], R, kind="ExternalInput")  # pi-permuted
    wk = nc.dram_tensor("wk", [C, D], R, kind="ExternalInput")  # pi-permuted, head-avg
    wv = nc.dram_tensor("wv", [C, D], R, kind="ExternalInput")  # head-avg
    cosh = nc.dram_tensor("cosh", [T, HD], fp32, kind="ExternalInput")
    sinh = nc.dram_tensor("sinh", [T, HD], fp32, kind="ExternalInput")
    masks = nc.dram_tensor("masks", [P, 4 * TS], R, kind="ExternalInput")
    ident = nc.dram_tensor("ident", [P, P], R, kind="ExternalInput")
    ones = nc.dram_tensor("ones", [P, 1], R, kind="ExternalInput")
    oT = nc.dram_tensor("oT", [D, T], fp32, kind="ExternalOutput")

    scale = float(D) ** -0.5

    with TileContext(nc) as tc:
        with tc.tile_pool(name="persist", bufs=1) as pp:
            qT_sb = pp.tile([P, NDT, T], R)
            kT_sb = pp.tile([P, NDT, T], R)
            v_sb = pp.tile([P, NT, D], R)

            # ---------------- phase 1: projections + rope + transposes ----
            with (
                tc.tile_pool(name="wpool", bufs=1) as wp,
                tc.tile_pool(name="xs", bufs=3) as sp,
                tc.tile_pool(name="tab", bufs=2) as tb,
                tc.tile_pool(name="rope", bufs=2) as rp,
                tc.tile_pool(name="ps_proj", bufs=2, space="PSUM") as ps1,
                tc.tile_pool(name="ps_tp", bufs=2, space="PSUM") as pst,
            ):
                ident_sb = wp.tile([P, P], R)
                nc.sync.dma_start(out=ident_sb, in_=ident[:, :])
                wq_sb = wp.tile([P, NCC, D], R)
                wk_sb = wp.tile([P, NCC, D], R)
                wv_sb = wp.tile([P, NCC, D], R)
                # per-chunk interleaved weight stream: chunk c's matmuls can
                # start as soon as chunk c lands (v first, then k, then q —
                # matching per-chunk matmul order below)
                for c in range(NCC):
                    cs = slice(c * P, (c + 1) * P)
                    nc.sync.dma_start(out=wv_sb[:, c, :], in_=wv[cs, :])
                    nc.sync.dma_start(out=wk_sb[:, c, :], in_=wk[cs, :])
                    nc.sync.dma_start(out=wq_sb[:, c, :], in_=wq[cs, :])

                def consume(tt, vp, kp, qp, cos_t, sin_t):
                    # v: copy to resident natural layout (Act, rounds to tf32)
                    nc.scalar.copy(v_sb[:, tt, :], vp)
                    # all PSUM-releasing rope muls first, then transposes,
                    # then the qT/kT evacuation copies — keeps PSUM slot
                    # turnaround off the PE's critical path
                    chains = []
                    for src in (kp, qp):
                        lo, hi = src[:, 0:HD], src[:, HD:D]
                        tmp = rp.tile([P, D], fp32, tag="rtmp", name="tmp")
                        qc = rp.tile([P, D], R, tag="rcos", name="qc")
                        nc.vector.tensor_mul(tmp[:, 0:HD], hi, sin_t)
                        nc.vector.tensor_mul(tmp[:, HD:D], lo, sin_t)
                        nc.vector.tensor_mul(qc[:, 0:HD], lo, cos_t)
                        nc.vector.tensor_mul(qc[:, HD:D], hi, cos_t)
                        nc.vector.tensor_sub(qc[:, 0:HD], qc[:, 0:HD], tmp[:, 0:HD])
                        nc.vector.tensor_add(qc[:, HD:D], qc[:, HD:D], tmp[:, HD:D])
                        chains.append(qc)
                    for qc, dst in zip(chains, (kT_sb, qT_sb)):
                        tps = pst.tile([P, NDT, P], R, tag="tp", name="tps")
                        for dt in range(NDT):
                            nc.tensor.transpose(
                                tps[:, dt, :],
                                qc[:, dt * P : (dt + 1) * P],
                                ident_sb,
                            )
                        nc.vector.tensor_copy(
                            dst[:, :, tt * P : (tt + 1) * P], tps
                        )

                pend = None
                xcs = []
                for tt in range(NT):  # 16 t-tiles, x DMA'd in c-pairs
                    t0 = tt * P
                    # SP's queue is busy streaming weights during the first
                    # tiles — route their x/table DMAs via Act instead.
                    dq = nc.scalar if tt < 6 else nc.sync
                    cos_t = tb.tile([P, HD], fp32, tag="cos", name="cos_t")
                    sin_t = tb.tile([P, HD], fp32, tag="sin", name="sin_t")
                    vp = ps1.tile([P, D], fp32, tag="v", name="vp")
                    kp = ps1.tile([P, D], fp32, tag="k", name="kp")
                    qp = ps1.tile([P, D], fp32, tag="q", name="qp")
                    for c in range(NCC):
                        if c % 2 == 0:
                            xc = sp.tile([P, 2, P], R, tag="x", name="xc")
                            dq.dma_start(
                                out=xc,
                                in_=xT[c * P : (c + 2) * P, t0 : t0 + P].rearrange(
                                    "(two p) t -> p two t", p=P
                                ),
                            )
                            xcs.append(xc)
                        xsl = xcs[-1][:, c % 2, :]
                        if c == 2:
                            dq.dma_start(out=cos_t, in_=cosh[t0 : t0 + P, :])
                            dq.dma_start(out=sin_t, in_=sinh[t0 : t0 + P, :])
                        st, sn = (c == 0), (c == NCC - 1)
                        nc.tensor.matmul(vp, xsl, wv_sb[:, c, :], start=st, stop=sn)
                        nc.tensor.matmul(kp, xsl, wk_sb[:, c, :], start=st, stop=sn)
                        nc.tensor.matmul(qp, xsl, wq_sb[:, c, :], start=st, stop=sn)
                    if pend is not None:
                        consume(*pend)
                    pend = (tt, vp, kp, qp, cos_t, sin_t)
                consume(*pend)

            # ---------------- phase 2: causal attention -------------------
            with (
                tc.tile_pool(name="ps_sc", bufs=3, space="PSUM") as pssc,
                tc.tile_pool(name="ps_sum", bufs=1, space="PSUM") as pssum,
                tc.tile_pool(name="ps_av", bufs=4, space="PSUM") as psav,
                tc.tile_pool(name="expp", bufs=3) as ep,
                tc.tile_pool(name="avsp", bufs=4) as avp,
                tc.tile_pool(name="outp", bufs=3) as op_,
            ):
                mask_sb = op_.tile([P, 4 * TS], R, tag="masks", bufs=1)
                nc.sync.dma_start(out=mask_sb, in_=masks[:, :])
                ones_sb = op_.tile([P, 1], R, tag="ones", bufs=1)
                nc.sync.dma_start(out=ones_sb, in_=ones[:, :])
                for j in range(NSLAB):
                    tsl = slice(j * TS, (j + 1) * TS)
                    nst = (TS // P) * (j + 1)  # s-tiles needed (causal)
                    sums = pssum.tile([1, TS], fp32, tag="sum")
                    av = [psav.tile([P, TS], fp32, tag="av", name="av") for _ in range(NDT)]
                    for st in range(nst):
                        m = st - (TS // P) * j
                        # diagonal-block tiles only need columns right of the
                        # causal boundary; fp32r needs >=256-wide moving dim,
                        # so clamp the restriction to a 256-wide floor
                        c0 = 0 if m < 1 else min(m * P, TS - 2 * P)
                        csl = slice(c0, TS)
                        tcl = slice(j * TS + c0, (j + 1) * TS)
                        sc = pssc.tile([P, TS], fp32, tag="sc")
                        for dt in range(NDT):
                            nc.tensor.matmul(
                                sc[:, csl],
                                kT_sb[:, dt, st * P : (st + 1) * P],
                                qT_sb[:, dt, tcl],
                                start=(dt == 0),
                                stop=(dt == NDT - 1),
                            )
                        expt = ep.tile([P, TS], R, tag="exp")
                        nc.scalar.activation(
                            out=expt[:, csl], in_=sc[:, csl], func=EXP, scale=scale
                        )
                        if m >= 0:  # diagonal tile: zero the s > t part
                            nc.vector.tensor_mul(
                                expt[:, csl],
                                expt[:, csl],
                                mask_sb[:, m * TS + c0 : (m + 1) * TS],
                            )
                        nc.tensor.matmul(
                            sums[:, csl],
                            ones_sb,
                            expt[:, csl],
                            start=(st == 0),
                            stop=(st == nst - 1),
                            skip_group_check=True,
                        )
                        for dt in range(NDT):
                            nc.tensor.matmul(
                                av[dt][:, csl],
                                v_sb[:, st, dt * P : (dt + 1) * P],
                                expt[:, csl],
                                start=(st == 0),
                                stop=(st == nst - 1),
                                skip_group_check=True,
                            )
                    # 1/sum chain first (needs only sums, which stops before
                    # the last av matmuls), then AV PSUM evacuation on Act so
                    # the next slab's accumulation never waits on it
                    rec = op_.tile([1, TS], fp32, tag="rec")
                    nc.vector.reciprocal(rec, sums)
                    recb = op_.tile([P, TS], fp32, tag="recb", bufs=2)
                    nc.gpsimd.partition_broadcast(recb, rec)
                    avs = [None] * NDT
                    for dt in range(NDT):
                        avs[dt] = avp.tile([P, TS], fp32, tag="avs", name="avs")
                        nc.scalar.copy(avs[dt], av[dt])
                    for dt in range(NDT):
                        ob = op_.tile([P, TS], fp32, tag="ob", bufs=4)
                        nc.vector.tensor_mul(ob, avs[dt], recb)
                        nc.sync.dma_start(out=oT[dt * P : (dt + 1) * P, tsl], in_=ob)

    nc.finalize()
    return nc


def _tf32(a):
    """Round fp32 to tf32 (10-bit mantissa, round-to-nearest)."""
    b = np.ascontiguousarray(a, dtype=np.float32).view(np.uint32)
    r = ((b >> np.uint32(13)) + ((b >> np.uint32(12)) & np.uint32(1))) << np.uint32(13)
    return r.view(np.float32)


def _host_inputs(x, Wq, Wk, Wv):
    pi = np.concatenate([np.arange(0, D, 2), np.arange(1, D, 2)])
    f32 = np.float32

    wk_avg = Wk.mean(axis=0)  # [D, C]
    wv_avg = Wv.mean(axis=0)
    wk_p = _tf32(np.ascontiguousarray(wk_avg.T[:, pi], dtype=f32))
    wv_t = _tf32(np.ascontiguousarray(wv_avg.T, dtype=f32))

    freqs = 1.0 / (ROPE_THETA ** (np.arange(0, D, 2, dtype=np.float64) / D))
    ang = np.arange(T, dtype=np.float64)[:, None] * freqs[None, :]  # [T, D/2]
    cosb = np.cos(ang).astype(f32)
    sinb = np.sin(ang).astype(f32)

    m = np.zeros((P, 4 * TS), f32)
    for mi in range(4):
        s = np.arange(P)[:, None] + mi * P
        t = np.arange(TS)[None, :]
        m[:, mi * TS : (mi + 1) * TS] = (s <= t).astype(f32)

    ident = np.eye(P, dtype=f32)
    ones = np.ones((P, 1), f32)

    shared = {
        "wk": wk_p,
        "wv": wv_t,
        "cosh": cosb,
        "sinh": sinb,
        "masks": m,
        "ident": ident,
        "ones": ones,
    }
    in_maps = []
    for i in range(NCORES):
        b, h = i // H, i % H
        in_maps.append(
            {
                "xT": _tf32(np.ascontiguousarray(x[b].T, dtype=f32)),
                "wq": _tf32(np.ascontiguousarray(Wq[h].T[:, pi], dtype=f32)),
                **shared,
            }
        )
    return in_maps


def _run(x, Wq, Wk, Wv, trace=False):
    from concourse.bass_utils import run_bass_kernel_spmd

    if "nc" not in _CACHE:
        _CACHE["nc"] = _build()
    in_maps = _host_inputs(x, Wq, Wk, Wv)
    res = run_bass_kernel_spmd(
        _CACHE["nc"], in_maps, list(range(NCORES)), trace=trace
    )
    out = np.empty((B, H, T, D), np.float32)
    for i in range(NCORES):
        out[i // H, i % H] = res.results[i]["oT"].T
    return out.reshape(B, T, H * D), res


def kernel(**inputs):
    out, _ = _run(inputs["x"], inputs["Wq"], inputs["Wk"], inputs["Wv"])
    return out


# revision 26
# speedup vs baseline: 3.8956x; 1.1212x over previous
"""Grouped-query attention (2 query heads, 1 pooled KV head) with RoPE,
causal softmax — Trainium2 Bass/Tile kernel, 8 NeuronCores.

Sharding: one core per (batch, head) pair (4 x 2 = 8 cores). The pooled KV
head is head-averaged on the host (mean over heads commutes with the linear
projection), so each core does: q/k/v projections, RoPE on q/k, causal
attention.

Precision: all matmuls run in fp32r (tf32) — 1 PE cycle/row instead of
fp32's 4 — with fp32 PSUM accumulation. Inputs are tf32-rounded on the
host; on-device fp32r producers (rope DVE ops, exp activation, PSUM
evacuation copies) round at write time as the BIR verifier requires.

Layout strategy (all chosen so every DMA is a natural contiguous slice):
  - host passes x[b] transposed (xT [C, T]) so the contraction dim c lands on
    SBUF partitions for the projection matmuls (x chunks are the stationary
    operand, shared by q/k/v).
  - projections produce q/k/v in natural [t, d] layout; RoPE is applied there.
  - the head dim of q/k (and their weights) is permuted to "evens-then-odds"
    (pi) so the RoPE pair swap becomes a contiguous half-block swap along the
    free dim. The permutation cancels in the q.k^T contraction. RoPE tables
    are stored halved (cos/sin of [T, D/2]) with the sign handled by
    tensor_sub/tensor_add on the two halves.
  - rotated q/k are PE-transposed (fp32r, exact) to qT/kT [d, t]; scores are
    computed transposed (scoresT [s, t]) so softmax normalization can be
    folded in after the AV matmul, and wei never needs a transpose.
  - softmax skips the max-subtraction (scores are O(1) by construction);
    exp + scale fused into one ScalarE op; denominators via ones-vector
    matmul; 1/sum applied to the AV output.

Engine/queue split (DMAs serialize per issuing engine in the HW queues):
  - SP queue: identity, per-chunk weight stream, phase-2 masks/ones + output.
  - Act queue: x chunk stream + rope tables (+ v and av PSUM evacuations,
    exp activations).
  - DVE: rope math, qT/kT PSUM evacuations, mask muls, 1/sum scaling.
"""

import sys

sys.path.insert(0, "/opt/trn_rl_repo")

import numpy as np

B, T, C = 4, 2048, 2048
H, D = 2, 512
HD = D // 2
NCORES = 8
ROPE_THETA = 10000.0
P = 128
NT = T // P  # 16 t/s tiles of 128
NCC = C // P  # 16 contraction chunks of 128
NDT = D // P  # 4 head-dim tiles of 128
TS = 512  # t-slab width for attention phase
NSLAB = T // TS  # 4

_CACHE = {}


def _build():
    import concourse.bass as bass
    import concourse.mybir as mybir
    from concourse import bacc
    from concourse.tile import TileContext

    fp32 = mybir.dt.float32
    R = mybir.dt.float32r
    BF16 = mybir.dt.bfloat16
    EXP = mybir.ActivationFunctionType.Exp

    nc = bacc.Bacc()
    xT = nc.dram_tensor("xT", [C, T], BF16, kind="ExternalInput")
    wq = nc.dram_tensor("wq", [C, D], BF16, kind="ExternalInput")  # pi-permuted
    wk = nc.dram_tensor("wk", [C, D], BF16, kind="ExternalInput")  # pi-permuted, head-avg
    wv = nc.dram_tensor("wv", [C, D], BF16, kind="ExternalInput")  # head-avg
    cosh = nc.dram_tensor("cosh", [T, HD], fp32, kind="ExternalInput")
    sinh = nc.dram_tensor("sinh", [T, HD], fp32, kind="ExternalInput")
    masks = nc.dram_tensor("masks", [P, 4 * TS], R, kind="ExternalInput")
    ident = nc.dram_tensor("ident", [P, P], R, kind="ExternalInput")
    ones = nc.dram_tensor("ones", [P, 1], R, kind="ExternalInput")
    oT = nc.dram_tensor("oT", [D, T], fp32, kind="ExternalOutput")

    scale = float(D) ** -0.5

    with TileContext(nc) as tc:
        with tc.tile_pool(name="persist", bufs=1) as pp:
            qT_sb = pp.tile([P, NDT, T], R)
            kT_sb = pp.tile([P, NDT, T], R)
            v_sb = pp.tile([P, NT, D], R)
            mask_sb = pp.tile([P, 4 * TS], R)
            ones_sb = pp.tile([P, 1], R)

            # ---------------- phase 1: projections + rope + transposes ----
            with (
                tc.tile_pool(name="wpool", bufs=1) as wp,
                tc.tile_pool(name="xs", bufs=3) as sp,
                tc.tile_pool(name="tab", bufs=2) as tb,
                tc.tile_pool(name="rope", bufs=2) as rp,
                tc.tile_pool(name="ps_proj", bufs=2, space="PSUM") as ps1,
                tc.tile_pool(name="ps_tp", bufs=2, space="PSUM") as pst,
            ):
                ident_sb = wp.tile([P, P], R)
                nc.sync.dma_start(out=ident_sb, in_=ident[:, :])
                wq_sb = wp.tile([P, NCC, D], BF16)
                wk_sb = wp.tile([P, NCC, D], BF16)
                wv_sb = wp.tile([P, NCC, D], BF16)
                # block-interleaved weight stream: chunk c's matmuls can
                # start as soon as its 4-chunk block lands (v first, then k,
                # then q — matching per-chunk matmul order below)
                for b0, nb in ((0, 4), (4, 4), (8, 4), (12, 4)):
                    bs = slice(b0 * P, (b0 + nb) * P)
                    for w_sb, w_dram in ((wv_sb, wv), (wk_sb, wk), (wq_sb, wq)):
                        nc.sync.dma_start(
                            out=w_sb[:, b0 : b0 + nb, :],
                            in_=w_dram[bs, :].rearrange("(cc p) d -> p cc d", p=P),
                        )
                nc.sync.dma_start(out=mask_sb, in_=masks[:, :])
                nc.sync.dma_start(out=ones_sb, in_=ones[:, :])

                def consume(tt, vp, kp, qp, cos_t, sin_t):
                    # v: copy to resident natural layout (Act, rounds to tf32)
                    nc.scalar.copy(v_sb[:, tt, :], vp)
                    # all PSUM-releasing rope muls first, then transposes,
                    # then the qT/kT evacuation copies — keeps PSUM slot
                    # turnaround off the PE's critical path
                    chains = []
                    for src in (kp, qp):
                        lo, hi = src[:, 0:HD], src[:, HD:D]
                        tmp = rp.tile([P, D], fp32, tag="rtmp", name="tmp")
                        qc = rp.tile([P, D], R, tag="rcos", name="qc")
                        nc.vector.tensor_mul(tmp[:, 0:HD], hi, sin_t)
                        nc.vector.tensor_mul(tmp[:, HD:D], lo, sin_t)
                        nc.vector.tensor_mul(qc[:, 0:HD], lo, cos_t)
                        nc.vector.tensor_mul(qc[:, HD:D], hi, cos_t)
                        nc.vector.tensor_sub(qc[:, 0:HD], qc[:, 0:HD], tmp[:, 0:HD])
                        nc.vector.tensor_add(qc[:, HD:D], qc[:, HD:D], tmp[:, HD:D])
                        chains.append(qc)
                    for qc, dst in zip(chains, (kT_sb, qT_sb)):
                        tps = pst.tile([P, NDT, P], R, tag="tp", name="tps")
                        for dt in range(NDT):
                            nc.tensor.transpose(
                                tps[:, dt, :],
                                qc[:, dt * P : (dt + 1) * P],
                                ident_sb,
                            )
                        nc.vector.tensor_copy(
                            dst[:, :, tt * P : (tt + 1) * P], tps
                        )

                pend = None
                xcs = [None] * NCC
                for tt in range(NT):  # 16 t-tiles; x pieces span 2 t-tiles
                    t0 = tt * P
                    # SP's queue is busy streaming weights during the first
                    # tiles — route their x/table DMAs via Act instead.
                    dq = nc.scalar if tt < 6 else nc.sync
                    cos_t = tb.tile([P, HD], fp32, tag="cos", name="cos_t")
                    sin_t = tb.tile([P, HD], fp32, tag="sin", name="sin_t")
                    vp = ps1.tile([P, D], fp32, tag="v", name="vp")
                    kp = ps1.tile([P, D], fp32, tag="k", name="kp")
                    qp = ps1.tile([P, D], fp32, tag="q", name="qp")
                    for c in range(NCC):
                        if tt % 2 == 0 and c % 2 == 0:
                            # 2 c-chunks per DMA; 256-wide bf16 runs keep the
                            # transfers full-rate
                            xc = sp.tile([P, 2, 256], BF16, tag="x", name="xc", bufs=10)
                            dq.dma_start(
                                out=xc,
                                in_=xT[c * P : (c + 2) * P, t0 : t0 + 256].rearrange(
                                    "(two p) t -> p two t", p=P
                                ),
                            )
                            xcs[c // 2] = xc
                        xsl = xcs[c // 2][:, c % 2, (tt % 2) * P : (tt % 2 + 1) * P]
                        if c == 2:
                            dq.dma_start(out=cos_t, in_=cosh[t0 : t0 + P, :])
                            dq.dma_start(out=sin_t, in_=sinh[t0 : t0 + P, :])
                        st, sn = (c == 0), (c == NCC - 1)
                        nc.tensor.matmul(vp, xsl, wv_sb[:, c, :], start=st, stop=sn)
                        nc.tensor.matmul(kp, xsl, wk_sb[:, c, :], start=st, stop=sn)
                        nc.tensor.matmul(qp, xsl, wq_sb[:, c, :], start=st, stop=sn)
                    if pend is not None:
                        consume(*pend)
                    pend = (tt, vp, kp, qp, cos_t, sin_t)
                consume(*pend)

            # ---------------- phase 2: causal attention -------------------
            with (
                tc.tile_pool(name="ps_sc", bufs=3, space="PSUM") as pssc,
                tc.tile_pool(name="ps_sum", bufs=1, space="PSUM") as pssum,
                tc.tile_pool(name="ps_av", bufs=4, space="PSUM") as psav,
                tc.tile_pool(name="expp", bufs=3) as ep,
                tc.tile_pool(name="avsp", bufs=4) as avp,
                tc.tile_pool(name="outp", bufs=3) as op_,
            ):
                for j in range(NSLAB):
                    tsl = slice(j * TS, (j + 1) * TS)
                    nst = (TS // P) * (j + 1)  # s-tiles needed (causal)
                    sums = pssum.tile([1, TS], fp32, tag="sum")
                    av = [psav.tile([P, TS], fp32, tag="av", name="av") for _ in range(NDT)]

                    def accum(st, expt, csl):
                        # sums/av accumulation for s-tile st, issued one
                        # s-tile late so the exp->mask chain never blocks PE
                        nc.tensor.matmul(
                            sums[:, csl],
                            ones_sb,
                            expt[:, csl],
                            start=(st == 0),
                            stop=(st == nst - 1),
                            skip_group_check=True,
                        )
                        for dt in range(NDT):
                            nc.tensor.matmul(
                                av[dt][:, csl],
                                v_sb[:, st, dt * P : (dt + 1) * P],
                                expt[:, csl],
                                start=(st == 0),
                                stop=(st == nst - 1),
                                skip_group_check=True,
                            )

                    pend2 = None
                    for st in range(nst):
                        m = st - (TS // P) * j
                        # diagonal-block tiles only need columns right of the
                        # causal boundary; fp32r needs >=256-wide moving dim,
                        # so clamp the restriction to a 256-wide floor
                        c0 = 0 if m < 1 else min(m * P, TS - 2 * P)
                        csl = slice(c0, TS)
                        tcl = slice(j * TS + c0, (j + 1) * TS)
                        sc = pssc.tile([P, TS], fp32, tag="sc")
                        for dt in range(NDT):
                            nc.tensor.matmul(
                                sc[:, csl],
                                kT_sb[:, dt, st * P : (st + 1) * P],
                                qT_sb[:, dt, tcl],
                                start=(dt == 0),
                                stop=(dt == NDT - 1),
                            )
                        expt = ep.tile([P, TS], R, tag="exp")
                        nc.scalar.activation(
                            out=expt[:, csl], in_=sc[:, csl], func=EXP, scale=scale
                        )
                        if m >= 0:  # diagonal tile: zero the s > t part
                            nc.vector.tensor_mul(
                                expt[:, csl],
                                expt[:, csl],
                                mask_sb[:, m * TS + c0 : (m + 1) * TS],
                            )
                        if pend2 is not None:
                            accum(*pend2)
                        pend2 = (st, expt, csl)
                    accum(*pend2)
                    # 1/sum chain first (needs only sums, which stops before
                    # the last av matmuls), then AV PSUM evacuation on Act so
                    # the next slab's accumulation never waits on it
                    rec = op_.tile([1, TS], fp32, tag="rec")
                    nc.vector.reciprocal(rec, sums)
                    recb = op_.tile([P, TS], fp32, tag="recb", bufs=2)
                    nc.gpsimd.partition_broadcast(recb, rec)
                    avs = [None] * NDT
                    for dt in range(NDT):
                        avs[dt] = avp.tile([P, TS], fp32, tag="avs", name="avs")
                        nc.scalar.copy(avs[dt], av[dt])
                    for dt in range(NDT):
                        ob = op_.tile([P, TS], fp32, tag="ob", bufs=4)
                        nc.vector.tensor_mul(ob, avs[dt], recb)
                        nc.sync.dma_start(out=oT[dt * P : (dt + 1) * P, tsl], in_=ob)

    nc.finalize()
    return nc


def _tf32(a):
    """Round fp32 to tf32 (10-bit mantissa, round-to-nearest)."""
    b = np.ascontiguousarray(a, dtype=np.float32).view(np.uint32)
    r = ((b >> np.uint32(13)) + ((b >> np.uint32(12)) & np.uint32(1))) << np.uint32(13)
    return r.view(np.float32)


def _host_inputs(x, Wq, Wk, Wv):
    import ml_dtypes

    bf16 = ml_dtypes.bfloat16
    pi = np.concatenate([np.arange(0, D, 2), np.arange(1, D, 2)])
    f32 = np.float32

    wk_avg = Wk.mean(axis=0)  # [D, C]
    wv_avg = Wv.mean(axis=0)
    wk_p = np.ascontiguousarray(wk_avg.T[:, pi], dtype=f32).astype(bf16)
    wv_t = np.ascontiguousarray(wv_avg.T, dtype=f32).astype(bf16)

    freqs = 1.0 / (ROPE_THETA ** (np.arange(0, D, 2, dtype=np.float64) / D))
    ang = np.arange(T, dtype=np.float64)[:, None] * freqs[None, :]  # [T, D/2]
    cosb = np.cos(ang).astype(f32)
    sinb = np.sin(ang).astype(f32)

    m = np.zeros((P, 4 * TS), f32)
    for mi in range(4):
        s = np.arange(P)[:, None] + mi * P
        t = np.arange(TS)[None, :]
        m[:, mi * TS : (mi + 1) * TS] = (s <= t).astype(f32)

    ident = np.eye(P, dtype=f32)
    ones = np.ones((P, 1), f32)

    shared = {
        "wk": wk_p,
        "wv": wv_t,
        "cosh": cosb,
        "sinh": sinb,
        "masks": m,
        "ident": ident,
        "ones": ones,
    }
    in_maps = []
    for i in range(NCORES):
        b, h = i // H, i % H
        in_maps.append(
            {
                "xT": np.ascontiguousarray(x[b].T, dtype=f32).astype(bf16),
                "wq": np.ascontiguousarray(Wq[h].T[:, pi], dtype=f32).astype(bf16),
                **shared,
            }
        )
    return in_maps


def _run(x, Wq, Wk, Wv, trace=False):
    from concourse.bass_utils import run_bass_kernel_spmd

    if "nc" not in _CACHE:
        _CACHE["nc"] = _build()
    in_maps = _host_inputs(x, Wq, Wk, Wv)
    res = run_bass_kernel_spmd(
        _CACHE["nc"], in_maps, list(range(NCORES)), trace=trace
    )
    out = np.empty((B, H, T, D), np.float32)
    for i in range(NCORES):
        out[i // H, i % H] = res.results[i]["oT"].T
    return out.reshape(B, T, H * D), res


def kernel(**inputs):
    out, _ = _run(inputs["x"], inputs["Wq"], inputs["Wk"], inputs["Wv"])
    return out


# revision 42
# speedup vs baseline: 4.0213x; 1.0323x over previous
"""Grouped-query attention (2 query heads, 1 pooled KV head) with RoPE,
causal softmax — Trainium2 Bass/Tile kernel, 8 NeuronCores.

Sharding: one core per (batch, head) pair (4 x 2 = 8 cores). The pooled KV
head is head-averaged on the host (mean over heads commutes with the linear
projection), so each core does: q/k/v projections, RoPE on q/k, causal
attention.

Precision: all matmuls run in fp32r (tf32) — 1 PE cycle/row instead of
fp32's 4 — with fp32 PSUM accumulation. Inputs are tf32-rounded on the
host; on-device fp32r producers (rope DVE ops, exp activation, PSUM
evacuation copies) round at write time as the BIR verifier requires.

Layout strategy (all chosen so every DMA is a natural contiguous slice):
  - host passes x[b] transposed (xT [C, T]) so the contraction dim c lands on
    SBUF partitions for the projection matmuls (x chunks are the stationary
    operand, shared by q/k/v).
  - projections produce q/k/v in natural [t, d] layout; RoPE is applied there.
  - the head dim of q/k (and their weights) is permuted to "evens-then-odds"
    (pi) so the RoPE pair swap becomes a contiguous half-block swap along the
    free dim. The permutation cancels in the q.k^T contraction. RoPE tables
    are stored halved (cos/sin of [T, D/2]) with the sign handled by
    tensor_sub/tensor_add on the two halves.
  - rotated q/k are PE-transposed (fp32r, exact) to qT/kT [d, t]; scores are
    computed transposed (scoresT [s, t]) so softmax normalization can be
    folded in after the AV matmul, and wei never needs a transpose.
  - softmax skips the max-subtraction (scores are O(1) by construction);
    exp + scale fused into one ScalarE op; denominators via ones-vector
    matmul; 1/sum applied to the AV output.

Engine/queue split (DMAs serialize per issuing engine in the HW queues):
  - SP queue: identity, per-chunk weight stream, phase-2 masks/ones + output.
  - Act queue: x chunk stream + rope tables (+ v and av PSUM evacuations,
    exp activations).
  - DVE: rope math, qT/kT PSUM evacuations, mask muls, 1/sum scaling.
"""

import sys

sys.path.insert(0, "/opt/trn_rl_repo")

import numpy as np

B, T, C = 4, 2048, 2048
H, D = 2, 512
HD = D // 2
NCORES = 8
ROPE_THETA = 10000.0
P = 128
NT = T // P  # 16 t/s tiles of 128
NCC = C // P  # 16 contraction chunks of 128
NDT = D // P  # 4 head-dim tiles of 128
TS = 512  # t-slab width for attention phase
NSLAB = T // TS  # 4

_CACHE = {}


def _build():
    import concourse.bass as bass
    import concourse.mybir as mybir
    from concourse import bacc
    from concourse.tile import TileContext

    fp32 = mybir.dt.float32
    R = mybir.dt.float32r
    BF16 = mybir.dt.bfloat16
    EXP = mybir.ActivationFunctionType.Exp

    nc = bacc.Bacc()
    xT = nc.dram_tensor("xT", [C, T], BF16, kind="ExternalInput")
    wq = nc.dram_tensor("wq", [C, D], BF16, kind="ExternalInput")  # pi-permuted
    wk = nc.dram_tensor("wk", [C, D], BF16, kind="ExternalInput")  # pi-permuted, head-avg
    wv = nc.dram_tensor("wv", [C, D], BF16, kind="ExternalInput")  # head-avg
    cosh = nc.dram_tensor("cosh", [T, HD], fp32, kind="ExternalInput")
    sinh = nc.dram_tensor("sinh", [T, HD], fp32, kind="ExternalInput")
    masks = nc.dram_tensor("masks", [P, 4 * TS], BF16, kind="ExternalInput")
    ident = nc.dram_tensor("ident", [P, P], BF16, kind="ExternalInput")
    ones = nc.dram_tensor("ones", [P, 1], BF16, kind="ExternalInput")
    oT = nc.dram_tensor("oT", [D, T], fp32, kind="ExternalOutput")

    scale = float(D) ** -0.5

    with TileContext(nc) as tc:
        with tc.tile_pool(name="persist", bufs=1) as pp:
            qT_sb = pp.tile([P, NDT, T], BF16)
            kT_sb = pp.tile([P, NDT, T], BF16)
            v_sb = pp.tile([P, NT, D], BF16)
            mask_sb = pp.tile([P, 4 * TS], BF16)
            ones_sb = pp.tile([P, 1], BF16)

            # ---------------- phase 1: projections + rope + transposes ----
            with (
                tc.tile_pool(name="wpool", bufs=1) as wp,
                tc.tile_pool(name="xs", bufs=3) as sp,
                tc.tile_pool(name="tab", bufs=2) as tb,
                tc.tile_pool(name="rope", bufs=2) as rp,
                tc.tile_pool(name="ps_proj", bufs=2, space="PSUM") as ps1,
                tc.tile_pool(name="ps_tp", bufs=2, space="PSUM") as pst,
            ):
                ident_sb = wp.tile([P, P], BF16)
                nc.sync.dma_start(out=ident_sb, in_=ident[:, :])
                wq_sb = wp.tile([P, NCC, D], BF16)
                wk_sb = wp.tile([P, NCC, D], BF16)
                wv_sb = wp.tile([P, NCC, D], BF16)
                # block-interleaved weight stream: chunk c's matmuls can
                # start as soon as its 4-chunk block lands (v first, then k,
                # then q — matching per-chunk matmul order below)
                for b0, nb in ((0, 4), (4, 4), (8, 4), (12, 4)):
                    bs = slice(b0 * P, (b0 + nb) * P)
                    for w_sb, w_dram in ((wv_sb, wv), (wk_sb, wk), (wq_sb, wq)):
                        nc.sync.dma_start(
                            out=w_sb[:, b0 : b0 + nb, :],
                            in_=w_dram[bs, :].rearrange("(cc p) d -> p cc d", p=P),
                        )
                nc.sync.dma_start(out=mask_sb, in_=masks[:, :])
                nc.sync.dma_start(out=ones_sb, in_=ones[:, :])

                def consume(tt, vp, kp, qp, cos_t, sin_t, fine=False):
                    # v: copy to resident natural layout (Act, rounds to tf32)
                    nc.scalar.copy(v_sb[:, tt, :], vp)
                    # all PSUM-releasing rope muls first, then transposes,
                    # then the qT/kT evacuation copies — keeps PSUM slot
                    # turnaround off the PE's critical path. fine=True (last
                    # tile, nothing left to overlap with) interleaves each
                    # half's rope with its transposes to cut exposed latency.
                    chains = []
                    for src in (kp, qp):
                        lo, hi = src[:, 0:HD], src[:, HD:D]
                        tmp = rp.tile([P, D], fp32, tag="rtmp", name="tmp")
                        qc = rp.tile([P, D], BF16, tag="rcos", name="qc")
                        tps = pst.tile([P, NDT, P], BF16, tag="tp", name="tps")
                        nc.vector.tensor_mul(tmp[:, 0:HD], hi, sin_t)
                        nc.vector.tensor_mul(qc[:, 0:HD], lo, cos_t)
                        nc.vector.tensor_sub(qc[:, 0:HD], qc[:, 0:HD], tmp[:, 0:HD])
                        if fine:
                            for dt in range(2):
                                nc.tensor.transpose(
                                    tps[:, dt, :], qc[:, dt * P : (dt + 1) * P], ident_sb
                                )
                        nc.vector.tensor_mul(tmp[:, HD:D], lo, sin_t)
                        nc.vector.tensor_mul(qc[:, HD:D], hi, cos_t)
                        nc.vector.tensor_add(qc[:, HD:D], qc[:, HD:D], tmp[:, HD:D])
                        if fine:
                            for dt in range(2, NDT):
                                nc.tensor.transpose(
                                    tps[:, dt, :], qc[:, dt * P : (dt + 1) * P], ident_sb
                                )
                        chains.append((qc, tps))
                    for (qc, tps), dst in zip(chains, (kT_sb, qT_sb)):
                        if not fine:
                            for dt in range(NDT):
                                nc.tensor.transpose(
                                    tps[:, dt, :],
                                    qc[:, dt * P : (dt + 1) * P],
                                    ident_sb,
                                )
                        nc.vector.tensor_copy(
                            dst[:, :, tt * P : (tt + 1) * P], tps
                        )

                pend = None
                xcs = [None] * NCC
                # pair order chosen so the LAST t-tile pair processed (4,5)
                # is one phase 2 only needs ~18us in (slab 1) — hides the
                # final rope/transpose chain behind slab 0's attention
                tts = list(range(NT))
                for ti, tt in enumerate(tts):  # x pieces span each tile pair
                    t0 = tt * P
                    # SP's queue is busy streaming weights during the first
                    # tiles — route their x/table DMAs via Act instead.
                    dq = nc.scalar if ti < 6 else nc.sync
                    cos_t = tb.tile([P, HD], fp32, tag="cos", name="cos_t")
                    sin_t = tb.tile([P, HD], fp32, tag="sin", name="sin_t")
                    vp = ps1.tile([P, D], fp32, tag="v", name="vp")
                    kp = ps1.tile([P, D], fp32, tag="k", name="kp")
                    qp = ps1.tile([P, D], fp32, tag="q", name="qp")
                    for c in range(NCC):
                        if tt % 2 == 0 and c % 2 == 0:
                            # 2 c-chunks per DMA; 256-wide bf16 runs keep the
                            # transfers full-rate
                            xc = sp.tile([P, 2, 256], BF16, tag="x", name="xc", bufs=10)
                            dq.dma_start(
                                out=xc,
                                in_=xT[c * P : (c + 2) * P, t0 : t0 + 256].rearrange(
                                    "(two p) t -> p two t", p=P
                                ),
                            )
                            xcs[c // 2] = xc
                        xsl = xcs[c // 2][:, c % 2, (tt % 2) * P : (tt % 2 + 1) * P]
                        if c == 2:
                            dq.dma_start(out=cos_t, in_=cosh[t0 : t0 + P, :])
                            dq.dma_start(out=sin_t, in_=sinh[t0 : t0 + P, :])
                        st, sn = (c == 0), (c == NCC - 1)
                        nc.tensor.matmul(vp, xsl, wv_sb[:, c, :], start=st, stop=sn)
                        nc.tensor.matmul(kp, xsl, wk_sb[:, c, :], start=st, stop=sn)
                        nc.tensor.matmul(qp, xsl, wq_sb[:, c, :], start=st, stop=sn)
                    if pend is not None:
                        consume(*pend)
                    pend = (tt, vp, kp, qp, cos_t, sin_t)
                consume(*pend)

            # ---------------- phase 2: causal attention -------------------
            with (
                tc.tile_pool(name="ps_sc", bufs=3, space="PSUM") as pssc,
                tc.tile_pool(name="ps_sum", bufs=1, space="PSUM") as pssum,
                tc.tile_pool(name="ps_av", bufs=4, space="PSUM") as psav,
                tc.tile_pool(name="expp", bufs=3) as ep,
                tc.tile_pool(name="avsp", bufs=4) as avp,
                tc.tile_pool(name="outp", bufs=3) as op_,
            ):
                # last 512-slab split into column halves so its epilogue
                # mostly overlaps compute instead of trailing the kernel;
                # sub-slabs of one 512-group share a sums tile (disjoint
                # column ranges) so the bufs=1 slot never blocks them
                slabs = [(0, TS), (TS, TS), (2 * TS, TS), (3 * TS, TS)]
                sums = None
                for col0, width in slabs:
                    tsl = slice(col0, col0 + width)
                    soff = col0 % TS
                    ssl = slice(soff, soff + width)
                    nst = (col0 + width) // P  # s-tiles needed (causal)
                    if soff == 0:
                        sums = pssum.tile([1, TS], fp32, tag="sum")
                    av = [psav.tile([P, TS], fp32, tag="av", name="av") for _ in range(NDT)]

                    def accum(st, expt, csl):
                        # sums/av accumulation for s-tile st, issued one
                        # s-tile late so the exp->mask chain never blocks PE.
                        # av first: at slab start the sums matmul can still be
                        # waiting on the previous slab's 1/sum read. On the
                        # closing s-tile, sums first so 1/sum starts earlier.
                        def do_sums():
                            nc.tensor.matmul(
                                sums[:, soff + csl.start : soff + csl.stop],
                                ones_sb,
                                expt[:, csl],
                                start=(st == 0),
                                stop=(st == nst - 1),
                                skip_group_check=True,
                            )

                        if st == nst - 1:
                            do_sums()
                        for dt in range(NDT):
                            nc.tensor.matmul(
                                av[dt][:, csl],
                                v_sb[:, st, dt * P : (dt + 1) * P],
                                expt[:, csl],
                                start=(st == 0),
                                stop=(st == nst - 1),
                                skip_group_check=True,
                            )
                        if st != nst - 1:
                            do_sums()

                    pend2 = None
                    for st in range(nst):
                        rel = st * P - col0
                        # diagonal-block tiles only need columns right of the
                        # causal boundary
                        c0 = 0 if rel < P else rel
                        csl = slice(c0, width)
                        tcl = slice(col0 + c0, col0 + width)
                        sc = pssc.tile([P, TS], fp32, tag="sc")
                        for dt in range(NDT):
                            nc.tensor.matmul(
                                sc[:, csl],
                                kT_sb[:, dt, st * P : (st + 1) * P],
                                qT_sb[:, dt, tcl],
                                start=(dt == 0),
                                stop=(dt == NDT - 1),
                            )
                        expt = ep.tile([P, TS], BF16, tag="exp")
                        nc.scalar.activation(
                            out=expt[:, csl], in_=sc[:, csl], func=EXP, scale=scale
                        )
                        if rel >= 0:  # diagonal tile: zero the s > t part
                            nc.vector.tensor_mul(
                                expt[:, csl],
                                expt[:, csl],
                                mask_sb[:, (rel // P) * TS + c0 : (rel // P) * TS + width],
                            )
                        if pend2 is not None:
                            accum(*pend2)
                        pend2 = (st, expt, csl)
                    accum(*pend2)
                    # 1/sum chain first (needs only sums, which stops before
                    # the last av matmuls), then AV PSUM evacuation on Act so
                    # the next slab's accumulation never waits on it
                    rec = op_.tile([1, TS], fp32, tag="rec")
                    nc.vector.reciprocal(rec[:, :width], sums[:, ssl])
                    recb = op_.tile([P, TS], fp32, tag="recb", bufs=2)
                    nc.gpsimd.partition_broadcast(recb[:, :width], rec[:, :width])
                    last_slab = col0 + width == T
                    if not last_slab:
                        avs = [None] * NDT
                        for dt in range(NDT):
                            avs[dt] = avp.tile([P, TS], fp32, tag="avs", name="avs")
                            nc.scalar.copy(avs[dt][:, :width], av[dt][:, :width])
                    else:
                        # nothing reuses the AV banks after the final slab —
                        # scale straight out of PSUM, skipping the evacuation
                        avs = av
                    for dt in range(NDT):
                        ob = op_.tile([P, TS], fp32, tag="ob", bufs=4)
                        nc.vector.tensor_mul(ob[:, :width], avs[dt][:, :width], recb[:, :width])
                        nc.sync.dma_start(out=oT[dt * P : (dt + 1) * P, tsl], in_=ob[:, :width])

    nc.finalize()
    return nc


def _tf32(a):
    """Round fp32 to tf32 (10-bit mantissa, round-to-nearest)."""
    b = np.ascontiguousarray(a, dtype=np.float32).view(np.uint32)
    r = ((b >> np.uint32(13)) + ((b >> np.uint32(12)) & np.uint32(1))) << np.uint32(13)
    return r.view(np.float32)


def _host_inputs(x, Wq, Wk, Wv):
    import ml_dtypes

    bf16 = ml_dtypes.bfloat16
    pi = np.concatenate([np.arange(0, D, 2), np.arange(1, D, 2)])
    f32 = np.float32

    wk_avg = Wk.mean(axis=0)  # [D, C]
    wv_avg = Wv.mean(axis=0)
    wk_p = np.ascontiguousarray(wk_avg.T[:, pi], dtype=f32).astype(bf16)
    wv_t = np.ascontiguousarray(wv_avg.T, dtype=f32).astype(bf16)

    freqs = 1.0 / (ROPE_THETA ** (np.arange(0, D, 2, dtype=np.float64) / D))
    ang = np.arange(T, dtype=np.float64)[:, None] * freqs[None, :]  # [T, D/2]
    cosb = np.cos(ang).astype(f32)
    sinb = np.sin(ang).astype(f32)

    m = np.zeros((P, 4 * TS), f32)
    for mi in range(4):
        s = np.arange(P)[:, None] + mi * P
        t = np.arange(TS)[None, :]
        m[:, mi * TS : (mi + 1) * TS] = (s <= t).astype(f32)

    ident = np.eye(P, dtype=f32)
    ones = np.ones((P, 1), f32)

    shared = {
        "wk": wk_p,
        "wv": wv_t,
        "cosh": cosb,
        "sinh": sinb,
        "masks": m.astype(bf16),
        "ident": ident.astype(bf16),
        "ones": ones.astype(bf16),
    }
    in_maps = []
    for i in range(NCORES):
        b, h = i // H, i % H
        in_maps.append(
            {
                "xT": np.ascontiguousarray(x[b].T, dtype=f32).astype(bf16),
                "wq": np.ascontiguousarray(Wq[h].T[:, pi], dtype=f32).astype(bf16),
                **shared,
            }
        )
    return in_maps


def _run(x, Wq, Wk, Wv, trace=False):
    from concourse.bass_utils import run_bass_kernel_spmd

    if "nc" not in _CACHE:
        _CACHE["nc"] = _build()
    in_maps = _host_inputs(x, Wq, Wk, Wv)
    res = run_bass_kernel_spmd(
        _CACHE["nc"], in_maps, list(range(NCORES)), trace=trace
    )
    out = np.empty((B, H, T, D), np.float32)
    for i in range(NCORES):
        out[i // H, i % H] = res.results[i]["oT"].T
    return out.reshape(B, T, H * D), res


def kernel(**inputs):
    out, _ = _run(inputs["x"], inputs["Wq"], inputs["Wk"], inputs["Wv"])
    return out


# revision 60
# speedup vs baseline: 4.2234x; 1.0502x over previous
"""Grouped-query attention (2 query heads, 1 pooled KV head) with RoPE,
causal softmax — Trainium2 Bass/Tile kernel, 8 NeuronCores.

Sharding: one core per (batch, head) pair (4 x 2 = 8 cores). The pooled KV
head is head-averaged on the host (mean over heads commutes with the linear
projection), so each core does: q/k/v projections, RoPE on q/k, causal
attention.

Precision: all matmuls run on bf16 operands (1 PE cycle/row instead of
fp32's 4) with fp32 PSUM accumulation and fp32 softmax normalization /
output. Inputs are pre-rounded to bf16 on the host; on-device bf16
producers (rope DVE ops, exp activation, PSUM evacuation copies) round
at write time.

Layout strategy (all chosen so every DMA is a natural contiguous slice):
  - host passes x[b] transposed (xT [C, T]) so the contraction dim c lands on
    SBUF partitions for the projection matmuls (x chunks are the stationary
    operand, shared by q/k/v).
  - projections produce q/k/v in natural [t, d] layout; RoPE is applied there.
  - the head dim of q/k (and their weights) is permuted to "evens-then-odds"
    (pi) so the RoPE pair swap becomes a contiguous half-block swap along the
    free dim. The permutation cancels in the q.k^T contraction. RoPE tables
    are stored halved (cos/sin of [T, D/2]) with the sign handled by
    tensor_sub/tensor_add on the two halves.
  - rotated q/k are PE-transposed (bf16, exact) to qT/kT [d, t]; scores are
    computed transposed (scoresT [s, t]), and AV runs in [t, d] output
    layout (expt slices stationary) so the causal structure is whole-matmul
    skips and the output lands in natural [T, D] with no transpose.
  - softmax skips the max-subtraction (scores are O(1) by construction);
    exp + scale fused into one ScalarE op; denominators are per-partition
    [128t, 1] one-row matmuls (all four columns one PSUM accumulation
    group — start_tensor_calc marks the whole 2KB zero region, per-element
    pending-zero initializes each column); each t-tile is finalized
    (1/sum, fused scale+evacuate, output DMA) as soon as its causal
    diagonal closes, so the kernel tail holds only the last tile.

Engine/queue split (DMAs serialize per issuing engine in the HW queues):
  - SP queue: identity, per-chunk weight stream, phase-2 masks/ones + output.
  - Act queue: x chunk stream + rope tables (+ v and av PSUM evacuations,
    exp activations).
  - DVE: rope math, qT/kT PSUM evacuations, mask muls, 1/sum scaling.
"""

import sys

sys.path.insert(0, "/opt/trn_rl_repo")

import numpy as np

B, T, C = 4, 2048, 2048
H, D = 2, 512
HD = D // 2
NCORES = 8
ROPE_THETA = 10000.0
P = 128
NT = T // P  # 16 t/s tiles of 128
NCC = C // P  # 16 contraction chunks of 128
NDT = D // P  # 4 head-dim tiles of 128
TS = 512  # t-slab width for attention phase
NSLAB = T // TS  # 4

_CACHE = {}


def _build():
    import concourse.bass as bass
    import concourse.mybir as mybir
    from concourse import bacc
    from concourse.tile import TileContext

    fp32 = mybir.dt.float32
    R = mybir.dt.float32r
    BF16 = mybir.dt.bfloat16
    EXP = mybir.ActivationFunctionType.Exp

    nc = bacc.Bacc()
    xT = nc.dram_tensor("xT", [C, T], BF16, kind="ExternalInput")
    wq = nc.dram_tensor("wq", [C, D], BF16, kind="ExternalInput")  # pi-permuted
    wk = nc.dram_tensor("wk", [C, D], BF16, kind="ExternalInput")  # pi-permuted, head-avg
    wv = nc.dram_tensor("wv", [C, D], BF16, kind="ExternalInput")  # head-avg
    cosh = nc.dram_tensor("cosh", [T, HD], fp32, kind="ExternalInput")
    sinh = nc.dram_tensor("sinh", [T, HD], fp32, kind="ExternalInput")
    masks = nc.dram_tensor("masks", [P, 4 * TS], BF16, kind="ExternalInput")
    ident = nc.dram_tensor("ident", [P, P], BF16, kind="ExternalInput")
    ones = nc.dram_tensor("ones", [P, 1], BF16, kind="ExternalInput")
    oT = nc.dram_tensor("oT", [D, T], fp32, kind="ExternalOutput")

    scale = float(D) ** -0.5

    with TileContext(nc) as tc:
        with tc.tile_pool(name="persist", bufs=1) as pp:
            qT_sb = pp.tile([P, NDT, T], BF16)
            kT_sb = pp.tile([P, NDT, T], BF16)
            v_sb = pp.tile([P, NT, D], BF16)
            mask_sb = pp.tile([P, 4 * TS], BF16)
            ones_sb = pp.tile([P, 1], BF16)

            # ---------------- phase 1: projections + rope + transposes ----
            with (
                tc.tile_pool(name="wpool", bufs=1) as wp,
                tc.tile_pool(name="xs", bufs=3) as sp,
                tc.tile_pool(name="tab", bufs=2) as tb,
                tc.tile_pool(name="rope", bufs=2) as rp,
                tc.tile_pool(name="ps_proj", bufs=2, space="PSUM") as ps1,
                tc.tile_pool(name="ps_tp", bufs=2, space="PSUM") as pst,
            ):
                ident_sb = wp.tile([P, P], BF16)
                nc.sync.dma_start(out=ident_sb, in_=ident[:, :])
                wq_sb = wp.tile([P, NCC, D], BF16)
                wk_sb = wp.tile([P, NCC, D], BF16)
                wv_sb = wp.tile([P, NCC, D], BF16)
                # block-interleaved weight stream: chunk c's matmuls can
                # start as soon as its 4-chunk block lands (v first, then k,
                # then q — matching per-chunk matmul order below)
                for b0, nb in ((0, 4), (4, 4), (8, 4), (12, 4)):
                    bs = slice(b0 * P, (b0 + nb) * P)
                    for w_sb, w_dram in ((wv_sb, wv), (wk_sb, wk), (wq_sb, wq)):
                        nc.sync.dma_start(
                            out=w_sb[:, b0 : b0 + nb, :],
                            in_=w_dram[bs, :].rearrange("(cc p) d -> p cc d", p=P),
                        )
                nc.sync.dma_start(out=mask_sb, in_=masks[:, :])
                nc.sync.dma_start(out=ones_sb, in_=ones[:, :])

                def consume(tt, vp, kp, qp, cos_t, sin_t, fine=False):
                    # v: copy to resident natural layout (Act, rounds to tf32)
                    nc.scalar.copy(v_sb[:, tt, :], vp)
                    # all PSUM-releasing rope muls first, then transposes,
                    # then the qT/kT evacuation copies — keeps PSUM slot
                    # turnaround off the PE's critical path. fine=True (last
                    # tile, nothing left to overlap with) interleaves each
                    # half's rope with its transposes to cut exposed latency.
                    chains = []
                    for src in (kp, qp):
                        lo, hi = src[:, 0:HD], src[:, HD:D]
                        tmp = rp.tile([P, D], fp32, tag="rtmp", name="tmp")
                        qc = rp.tile([P, D], BF16, tag="rcos", name="qc")
                        tps = pst.tile([P, NDT, P], BF16, tag="tp", name="tps")
                        nc.vector.tensor_mul(tmp[:, 0:HD], hi, sin_t)
                        nc.vector.tensor_mul(qc[:, 0:HD], lo, cos_t)
                        nc.vector.tensor_sub(qc[:, 0:HD], qc[:, 0:HD], tmp[:, 0:HD])
                        if fine:
                            for dt in range(2):
                                nc.tensor.transpose(
                                    tps[:, dt, :], qc[:, dt * P : (dt + 1) * P], ident_sb
                                )
                        nc.vector.tensor_mul(tmp[:, HD:D], lo, sin_t)
                        nc.vector.tensor_mul(qc[:, HD:D], hi, cos_t)
                        nc.vector.tensor_add(qc[:, HD:D], qc[:, HD:D], tmp[:, HD:D])
                        if fine:
                            for dt in range(2, NDT):
                                nc.tensor.transpose(
                                    tps[:, dt, :], qc[:, dt * P : (dt + 1) * P], ident_sb
                                )
                        chains.append((qc, tps))
                    for (qc, tps), dst in zip(chains, (kT_sb, qT_sb)):
                        if not fine:
                            for dt in range(NDT):
                                nc.tensor.transpose(
                                    tps[:, dt, :],
                                    qc[:, dt * P : (dt + 1) * P],
                                    ident_sb,
                                )
                        nc.scalar.copy(
                            dst[:, :, tt * P : (tt + 1) * P], tps
                        )

                pend = None
                xcs = [None] * NCC
                # pair order chosen so the LAST t-tile pair processed (4,5)
                # is one phase 2 only needs ~18us in (slab 1) — hides the
                # final rope/transpose chain behind slab 0's attention
                tts = list(range(NT))
                for ti, tt in enumerate(tts):  # x pieces span each tile pair
                    t0 = tt * P
                    # SP's queue is busy streaming weights during the first
                    # tiles — route their x/table DMAs via Act instead.
                    dq = nc.scalar if ti < 6 else nc.sync
                    cos_t = tb.tile([P, HD], fp32, tag="cos", name="cos_t")
                    sin_t = tb.tile([P, HD], fp32, tag="sin", name="sin_t")
                    vp = ps1.tile([P, D], fp32, tag="v", name="vp")
                    kp = ps1.tile([P, D], fp32, tag="k", name="kp")
                    qp = ps1.tile([P, D], fp32, tag="q", name="qp")
                    for c in range(NCC):
                        if tt % 2 == 0 and c % 2 == 0:
                            # 2 c-chunks per DMA; 256-wide bf16 runs keep the
                            # transfers full-rate
                            xc = sp.tile([P, 2, 256], BF16, tag="x", name="xc", bufs=10)
                            dq.dma_start(
                                out=xc,
                                in_=xT[c * P : (c + 2) * P, t0 : t0 + 256].rearrange(
                                    "(two p) t -> p two t", p=P
                                ),
                            )
                            xcs[c // 2] = xc
                        xsl = xcs[c // 2][:, c % 2, (tt % 2) * P : (tt % 2 + 1) * P]
                        if c == 2:
                            dq.dma_start(out=cos_t, in_=cosh[t0 : t0 + P, :])
                            dq.dma_start(out=sin_t, in_=sinh[t0 : t0 + P, :])
                        st, sn = (c == 0), (c == NCC - 1)
                        nc.tensor.matmul(vp, xsl, wv_sb[:, c, :], start=st, stop=sn)
                        nc.tensor.matmul(kp, xsl, wk_sb[:, c, :], start=st, stop=sn)
                        nc.tensor.matmul(qp, xsl, wq_sb[:, c, :], start=st, stop=sn)
                    if pend is not None:
                        consume(*pend)
                    pend = (tt, vp, kp, qp, cos_t, sin_t)
                consume(*pend)

            # ---------------- phase 2: causal attention -------------------
            with (
                tc.tile_pool(name="ps_sc", bufs=3, space="PSUM") as pssc,
                tc.tile_pool(name="ps_sum", bufs=1, space="PSUM") as pssum,
                tc.tile_pool(name="ps_av", bufs=4, space="PSUM") as psav,
                tc.tile_pool(name="expp", bufs=3) as ep,
                tc.tile_pool(name="avsp", bufs=4) as avp,
                tc.tile_pool(name="outp", bufs=3) as op_,
            ):
                # last 512-slab split into column halves so its epilogue
                # mostly overlaps compute instead of trailing the kernel;
                # sub-slabs of one 512-group share a sums tile (disjoint
                # column ranges) so the bufs=1 slot never blocks them
                slabs = [(0, TS), (TS, TS), (2 * TS, TS), (3 * TS, TS)]
                sums = None
                for col0, width in slabs:
                    tsl = slice(col0, col0 + width)
                    soff = col0 % TS
                    ssl = slice(soff, soff + width)
                    nst = (col0 + width) // P  # s-tiles needed (causal)
                    if soff == 0:
                        sums = pssum.tile([1, TS], fp32, tag="sum")
                    av = [psav.tile([P, TS], fp32, tag="av", name="av") for _ in range(NDT)]

                    def accum(st, expt, csl):
                        # sums/av accumulation for s-tile st, issued one
                        # s-tile late so the exp->mask chain never blocks PE.
                        # av first: at slab start the sums matmul can still be
                        # waiting on the previous slab's 1/sum read. On the
                        # closing s-tile, sums first so 1/sum starts earlier.
                        for dt in range(NDT):
                            nc.tensor.matmul(
                                av[dt][:, csl],
                                v_sb[:, st, dt * P : (dt + 1) * P],
                                expt[:, csl],
                                start=(st == 0),
                                stop=(st == nst - 1),
                                skip_group_check=True,
                            )
                        nc.tensor.matmul(
                            sums[:, soff + csl.start : soff + csl.stop],
                            ones_sb,
                            expt[:, csl],
                            start=(st == 0),
                            stop=(st == nst - 1),
                            skip_group_check=True,
                        )

                    pend2 = None
                    for st in range(nst):
                        rel = st * P - col0
                        # diagonal-block tiles only need columns right of the
                        # causal boundary
                        c0 = 0 if rel < P else rel
                        csl = slice(c0, width)
                        tcl = slice(col0 + c0, col0 + width)
                        sc = pssc.tile([P, TS], fp32, tag="sc")
                        for dt in range(NDT):
                            nc.tensor.matmul(
                                sc[:, csl],
                                kT_sb[:, dt, st * P : (st + 1) * P],
                                qT_sb[:, dt, tcl],
                                start=(dt == 0),
                                stop=(dt == NDT - 1),
                            )
                        expt = ep.tile([P, TS], BF16, tag="exp")
                        nc.scalar.activation(
                            out=expt[:, csl], in_=sc[:, csl], func=EXP, scale=scale
                        )
                        if rel >= 0:  # diagonal tile: zero the s > t part
                            nc.vector.tensor_mul(
                                expt[:, csl],
                                expt[:, csl],
                                mask_sb[:, (rel // P) * TS + c0 : (rel // P) * TS + width],
                            )
                        if pend2 is not None:
                            accum(*pend2)
                        pend2 = (st, expt, csl)
                    accum(*pend2)
                    # 1/sum chain first (needs only sums, which stops before
                    # the last av matmuls), then AV PSUM evacuation on Act so
                    # the next slab's accumulation never waits on it
                    rec = op_.tile([1, TS], fp32, tag="rec")
                    nc.vector.reciprocal(rec[:, :width], sums[:, ssl])
                    recb_t = op_.tile([P, TS], fp32, tag="recb", bufs=2)
                    nc.gpsimd.partition_broadcast(recb_t[:, :width], rec[:, :width])
                    recb = recb_t[:, :width]
                    avs = [None] * NDT
                    for dt in range(NDT):
                        avs[dt] = avp.tile([P, TS], fp32, tag="avs", name="avs")
                        nc.scalar.copy(avs[dt][:, :width], av[dt][:, :width])
                    for dt in range(NDT):
                        ob = op_.tile([P, TS], fp32, tag="ob", bufs=4)
                        nc.vector.tensor_mul(ob[:, :width], avs[dt][:, :width], recb)
                        nc.sync.dma_start(out=oT[dt * P : (dt + 1) * P, tsl], in_=ob[:, :width])

    nc.finalize()
    return nc


def _tf32(a):
    """Round fp32 to tf32 (10-bit mantissa, round-to-nearest)."""
    b = np.ascontiguousarray(a, dtype=np.float32).view(np.uint32)
    r = ((b >> np.uint32(13)) + ((b >> np.uint32(12)) & np.uint32(1))) << np.uint32(13)
    return r.view(np.float32)


def _host_inputs(x, Wq, Wk, Wv):
    import ml_dtypes

    bf16 = ml_dtypes.bfloat16
    pi = np.concatenate([np.arange(0, D, 2), np.arange(1, D, 2)])
    f32 = np.float32

    wk_avg = Wk.mean(axis=0)  # [D, C]
    wv_avg = Wv.mean(axis=0)
    wk_p = np.ascontiguousarray(wk_avg.T[:, pi], dtype=f32).astype(bf16)
    wv_t = np.ascontiguousarray(wv_avg.T, dtype=f32).astype(bf16)

    freqs = 1.0 / (ROPE_THETA ** (np.arange(0, D, 2, dtype=np.float64) / D))
    ang = np.arange(T, dtype=np.float64)[:, None] * freqs[None, :]  # [T, D/2]
    cosb = np.cos(ang).astype(f32)
    sinb = np.sin(ang).astype(f32)

    m = np.zeros((P, 4 * TS), f32)
    for mi in range(4):
        s = np.arange(P)[:, None] + mi * P
        t = np.arange(TS)[None, :]
        m[:, mi * TS : (mi + 1) * TS] = (s <= t).astype(f32)

    ident = np.eye(P, dtype=f32)
    ones = np.ones((P, 1), f32)

    shared = {
        "wk": wk_p,
        "wv": wv_t,
        "cosh": cosb,
        "sinh": sinb,
        "masks": m.astype(bf16),
        "ident": ident.astype(bf16),
        "ones": ones.astype(bf16),
    }
    in_maps = []
    for i in range(NCORES):
        b, h = i // H, i % H
        in_maps.append(
            {
                "xT": np.ascontiguousarray(x[b].T, dtype=f32).astype(bf16),
                "wq": np.ascontiguousarray(Wq[h].T[:, pi], dtype=f32).astype(bf16),
                **shared,
            }
        )
    return in_maps


def _run(x, Wq, Wk, Wv, trace=False):
    from concourse.bass_utils import run_bass_kernel_spmd

    if "nc" not in _CACHE:
        _CACHE["nc"] = _build()
    in_maps = _host_inputs(x, Wq, Wk, Wv)
    res = run_bass_kernel_spmd(
        _CACHE["nc"], in_maps, list(range(NCORES)), trace=trace
    )
    out = np.empty((B, H, T, D), np.float32)
    for i in range(NCORES):
        out[i // H, i % H] = res.results[i]["oT"].T
    return out.reshape(B, T, H * D), res


def kernel(**inputs):
    out, _ = _run(inputs["x"], inputs["Wq"], inputs["Wk"], inputs["Wv"])
    return out
